# revision 23
# baseline (speedup 1.0000x reference)
"""Trainium2 Bass kernel for nn_DataEmbedding_v2 — v7.

Data parallel over batch (2 per core).  Output stored f16 on device (gate is
rel 2e-2 vs absmax ~1e3; f16 adds ~5e-4) and upcast on host.  The tc scan
keeps v3's exact-f32 compare algebra (f16 hi/mid/lo splits accumulated
coarse-first into f32 psum).  v7 is built around the measured ~216 ns
per-matmul floor: fewer, wider matmuls, and no DRAM roundtrip for tau.

  - select: ONE N=256 matmul per tile — diag (K=6 rows @96:102) and one-hot
    gather (K=96 rows @0:96) share lhsT; rhs is a zero-padded [102,NT,2,128]
    table so the two halves land in separate psum columns
  - conv: A = taps1+2 (one K=128 matmul vs a col-aligned double image),
    B = tap0 + tau row (K=65) — tau reaches the B lhsT via PE transpose +
    SBUF->SBUF flatten DMA (no DRAM bounce), tc_w applied as single f16 row
  - masking: ACT Sign, DVE value-mults + max-reduces (DVE/GpSimd port
    sharing avoided; GpSimd only issues DMAs/broadcasts)
  - warmup + filler matmuls keep the PE clock un-throttled
"""

import math
import os
import sys

sys.path.insert(0, "/opt/trn_rl_repo")

import numpy as np

B, S, C, D = 16, 2048, 64, 512
NCORES = 8
BLOC = B // NCORES
P = 128
NT = S // P
ETA = 0.3
EPS = 1e-08
BIG = 60000.0

WARM_N = 9           # warmup matmuls (N=512)
FILL_N = 4           # filler matmuls before conv b0
OG = 8               # output tiles per DMA group
DRAIN = ["V", "A"] * (NT // 2)   # V: DVE fused(+pe16) / A: ACT copy (pe MM)

# blob16 column layout
B16_IDENT = 0
B16_DVAL = 128
B16_SVAL = B16_DVAL + NT * P
B16_MASK1 = B16_SVAL + P
B16_W = B16_MASK1 + NT * NT
# blob32 column layout
B32_IDENT = 0
B32_T1C = 128
B32_KCOL = B32_T1C + NT
B32_ONES = B32_KCOL + 1
B32_W = B32_ONES + P


def _emit(tc, aps):
    from contextlib import ExitStack

    from concourse import mybir

    f32 = mybir.dt.float32
    f16 = mybir.dt.float16
    Alu = mybir.AluOpType
    Ax = mybir.AxisListType

    nc = tc.nc
    xin, xta2, xtb, pe16 = aps["xin"], aps["xta2"], aps["xtb"], aps["pe16"]
    wt12, wtb = aps["wt12"], aps["wtb"]
    blob16, blob32 = aps["blob16"], aps["blob32"]
    out = aps["out"]
    js_dram = aps["js_dram"]

    with ExitStack() as ctx:
        singles = ctx.enter_context(tc.tile_pool(name="singles", bufs=1))
        xpool = ctx.enter_context(tc.tile_pool(name="xpool", bufs=2))
        xtp = ctx.enter_context(tc.tile_pool(name="xtp", bufs=2))
        small = ctx.enter_context(tc.tile_pool(name="small", bufs=2))
        selp = ctx.enter_context(tc.tile_pool(name="selp", bufs=2))
        sgp = ctx.enter_context(tc.tile_pool(name="sgp", bufs=3))
        outp = ctx.enter_context(tc.tile_pool(name="outp", bufs=2))
        psA = ctx.enter_context(tc.tile_pool(name="psA", bufs=4, space="PSUM"))
        psD = ctx.enter_context(tc.tile_pool(name="psD", bufs=2, space="PSUM"))

        # ---- early DMAs (xin b0 first, then blobs, all HWDGE) ----
        xin0 = xpool.tile([P, NT, C], f32, tag="xin", name="xin0")
        nc.scalar.dma_start(xin0, xin[0])
        b16 = singles.tile([P, B16_W], f16)
        nc.scalar.dma_start(b16, blob16)
        b32 = singles.tile([P, B32_W], f32)
        nc.scalar.dma_start(b32, blob32)
        identh = b16[:, B16_IDENT : B16_IDENT + P]
        dval = b16[:, B16_DVAL:B16_SVAL].rearrange("p (i u) -> p i u", i=NT)
        sval = b16[:, B16_SVAL:B16_MASK1]
        mask1 = b16[:, B16_MASK1:B16_W].rearrange("p (i j) -> p i j", i=NT)
        identf = b32[:, B32_IDENT : B32_IDENT + P]
        t1c = b32[:, B32_T1C : B32_T1C + NT]
        kcol96 = b32[:, B32_KCOL : B32_KCOL + 1]

        xins, xta2s, xtbs = {}, {}, {}
        xins[0] = xin0
        xin1 = xpool.tile([P, NT, C], f32, tag="xin", name="xin1")
        nc.scalar.dma_start(xin1, xin[1])
        xins[1] = xin1
        for b in range(BLOC):
            xta2_sb = xtp.tile([P, S + 2], f16, tag="xta2", name=f"xta2_{b}")
            nc.gpsimd.dma_start(xta2_sb, xta2[b])
            xta2s[b] = xta2_sb
            xtb_sb = xtp.tile([66, S + 2], f16, tag="xtb", name=f"xtb{b}")
            nc.gpsimd.dma_start(xtb_sb[0:C, :], xtb[b])
            xtbs[b] = xtb_sb

        wt12_sb = singles.tile([P, D], f16)
        nc.gpsimd.dma_start(wt12_sb, wt12)
        wtb_sb = singles.tile([66, D], f16)
        nc.gpsimd.dma_start(wtb_sb, wtb)
        pe_sb = singles.tile([P, NT, D], f16)
        for q in range(4):
            nc.gpsimd.dma_start(
                pe_sb[:, q * 4 : (q + 1) * 4, :],
                pe16[:, q * 4 : (q + 1) * 4, :],
            )

        # ---- T2z tables: one per batch, zeroed then patched ----
        t2zs = {}
        for b in range(BLOC):
            t2z = selp.tile([102, NT, 2, P], f16, tag="t2z", name=f"t2z{b}")
            nc.scalar.memzero(t2z)
            # row 16 (hi "none") = -BIG; rows 17/49/81 (q partners) = 1;
            # rows 97/99/101 (diag q partners) = 1
            nc.gpsimd.dma_start(
                t2z[16:17, :, 1, :],
                aps["t2zs"][0:1].rearrange("r (i u) -> r i u", i=NT),
            )
            nc.gpsimd.dma_start(
                t2z[17:82:32, :, 1, :],
                aps["t2zs"][1:4].rearrange("r (i u) -> r i u", i=NT),
            )
            nc.gpsimd.dma_start(
                t2z[97:102:2, :, 0, :],
                aps["t2zs"][4:7].rearrange("r (i u) -> r i u", i=NT),
            )
            t2zs[b] = t2z

        # ---- phase A, stage-interleaved across batches ----
        st = {}
        spls, negths = {}, {}
        for b in range(BLOC):
            xin_sb = xins[b]
            normc = small.tile([P, NT], f32, tag="normc", name=f"normc{b}")
            nc.vector.tensor_reduce(
                normc, xin_sb, axis=Ax.X, op=Alu.add, apply_absolute_value=True
            )
            negthc = small.tile([P, 2, NT], f32, tag="negthc", name=f"negthc{b}")
            nc.vector.tensor_scalar(negthc[:, 0, :], normc, -1.0, None, op0=Alu.mult)
            nc.vector.tensor_scalar(
                negthc[:, 1, :],
                normc,
                float(EPS),
                float(1.0 - ETA),
                op0=Alu.add,
                op1=Alu.mult,
            )
            spl = small.tile([P, 2, 3, NT], f16, tag="spl", name=f"spl{b}")
            nc.vector.tensor_copy(spl[:, :, 0, :], negthc)
            rem1 = small.tile([P, 2, NT], f32, tag="rem1", name=f"rem1_{b}")
            nc.vector.tensor_tensor(rem1, negthc, spl[:, :, 0, :], op=Alu.subtract)
            nc.vector.tensor_copy(spl[:, :, 1, :], rem1)
            rem2 = small.tile([P, 2, NT], f32, tag="rem2", name=f"rem2_{b}")
            nc.vector.tensor_tensor(rem2, rem1, spl[:, :, 1, :], op=Alu.subtract)
            nc.vector.tensor_copy(spl[:, :, 2, :], rem2)
            spls[b] = spl
            negths[b] = negthc
            ps96 = psD.tile([96, P], f16, tag="psd", name=f"ps96_{b}")
            nc.tensor.transpose(ps96, spl.rearrange("p a k j -> p (a k j)"), identh)
            spT = small.tile([96, P], f16, tag="spt", name=f"spT{b}")
            nc.scalar.copy(spT, ps96)
            psnT = psD.tile([NT, P], f32, tag="psd", name=f"psnT{b}")
            nc.tensor.transpose(psnT, negthc[:, 0, :], identf)
            bneg = small.tile([NT, 1], f32, tag="bneg", name=f"bneg{b}")
            nc.vector.tensor_reduce(bneg, psnT, axis=Ax.X, op=Alu.max)
            st[b] = [spT, bneg]
            if b == 0:
                # warmup burst rides the gap while batch 1 data lands
                for w in range(WARM_N):
                    psw = psA.tile([P, D], f32, tag="psa", name=f"warm{w}")
                    nc.tensor.matmul(
                        psw, lhsT=b16[:, 0:P], rhs=b16[:, 0:D],
                        start=True, stop=True,
                    )
        bnegbcs = {}
        for b in range(BLOC):
            spT, bneg = st[b]
            psb1 = psD.tile([1, NT], f32, tag="psd", name=f"psb1_{b}")
            nc.tensor.transpose(psb1, bneg, identf[0:NT, 0:NT])
            bnegrow = small.tile([1, NT], f32, tag="bnegrow", name=f"bnegrow{b}")
            nc.scalar.copy(bnegrow, psb1)
            psbb = psD.tile([P, NT], f32, tag="psd", name=f"psbb{b}")
            nc.tensor.matmul(
                psbb,
                lhsT=b32[0:1, B32_ONES : B32_ONES + P],
                rhs=bnegrow,
                start=True,
                stop=True,
            )
            bnegbc = small.tile([P, NT], f32, tag="bnegbc", name=f"bnegbc{b}")
            nc.scalar.copy(bnegbc, psbb)
            bnegbcs[b] = bnegbc
        for b in range(BLOC):
            spT, _ = st[b]
            negthc = negths[b]
            l1a = small.tile([P, NT, NT], f32, tag="l1a", name=f"l1a{b}")
            nc.vector.tensor_tensor(
                l1a,
                negthc[:, 1, :].unsqueeze(2).to_broadcast([P, NT, NT]),
                bnegbcs[b].unsqueeze(1).to_broadcast([P, NT, NT]),
                op=Alu.add,
            )
            l1c = small.tile([P, NT, NT], f16, tag="l1c", name=f"l1c{b}")
            nc.vector.scalar_tensor_tensor(
                l1c, l1a, 0.0, mask1, op0=Alu.is_gt, op1=Alu.mult
            )
            jp1 = small.tile([P, NT], f16, tag="jp1", name=f"jp1_{b}")
            nc.vector.tensor_reduce(jp1, l1c, axis=Ax.X, op=Alu.max)
            st[b] = (jp1, spT)
        for b in range(BLOC):
            jp1, spT = st[b]
            psj = psD.tile([NT, P], f16, tag="psd", name=f"psj{b}")
            nc.tensor.transpose(psj, jp1, identh)
            jpT = small.tile([NT, P], f16, tag="jpt", name=f"jpT{b}")
            nc.scalar.copy(jpT, psj)
            nc.sync.dma_start(js_dram[b].rearrange("(i p) -> i p", p=P), jpT)

        # ---- select setup per batch ----
        sel = {}
        for b in range(BLOC):
            jp1, spT = st[b]
            t1 = selp.tile([102, S], f16, tag="t1", name=f"t1_{b}")
            ohsrc = selp.tile([96, S], f16, tag="ohsrc", name=f"ohsrc{b}")
            nc.gpsimd.dma_start(ohsrc, js_dram[b].partition_broadcast(96))
            nc.vector.tensor_scalar(
                t1[0:96, :], ohsrc, kcol96[0:96], None, op0=Alu.is_equal
            )
            for k in range(3):
                nc.scalar.dma_start(
                    t1[17 + 32 * k : 18 + 32 * k, :],
                    spT[48 + 16 * k : 64 + 16 * k],
                )
            nc.scalar.dma_start(t1[96:102, :], aps["dtmpl"])
            for k in range(3):
                nc.scalar.dma_start(
                    t1[97 + 2 * k : 98 + 2 * k, :],
                    spT[48 + 16 * k : 64 + 16 * k],
                )
            t2z = t2zs[b]
            for g in range(3):
                nc.gpsimd.dma_start(
                    t2z[g * 32 : g * 32 + 16, :, 1, :],
                    spT[g * 16 : (g + 1) * 16]
                    .unsqueeze(1)
                    .to_broadcast([16, NT, P]),
                )
            for k in range(3):
                nc.gpsimd.dma_start(
                    t2z[96 + 2 * k : 97 + 2 * k, :, 0, :],
                    spT[16 * k : 16 * (k + 1)],
                )
            sel[b] = (t1, t2z)

        for b in range(BLOC):
            t1, t2z = sel[b]
            nc.tensor.ldweights(t1[:, 0:P])
            nc.tensor.ldweights(t2z[:, 0, 0, :])

        for w in range(10):
            psw = psA.tile([P, D], f32, tag="psa", name=f"fillb{w}")
            nc.tensor.matmul(
                psw, lhsT=b16[:, 0:P], rhs=b16[:, 0:D], start=True, stop=True
            )

        # ---- select matmuls + sign/value/max per batch; tau chain ----
        for b in range(BLOC):
            t1, t2z = sel[b]
            xtb_sb = xtbs[b]
            rdiag = small.tile([P, NT], f16, tag="rdiag", name=f"rdiag{b}")
            rsel = small.tile([P, NT], f16, tag="rsel", name=f"rsel{b}")
            for h in range(4):
                psd = psD.tile([P, 4, 2, P], f32, tag="psd", name=f"psd{b}_{h}")
                for ii in range(4):
                    i = h * 4 + ii
                    sl = slice(i * P, (i + 1) * P)
                    nc.tensor.matmul(
                        psd[:, ii, :, :],
                        lhsT=t1[:, sl],
                        rhs=t2z[:, i, :, :],
                        start=True,
                        stop=True,
                    )
                sgn = sgp.tile([P, 4, 2, P], f16, tag="sgn", name=f"sgn{b}_{h}")
                nc.scalar.sign(sgn, psd)
                dm = sgp.tile([P, 4, P], f16, tag="dm", name=f"dm{b}_{h}")
                nc.vector.tensor_tensor(
                    dm, sgn[:, :, 0, :], dval[:, h * 4 : (h + 1) * 4, :],
                    op=Alu.mult,
                )
                sm = sgp.tile([P, 4, P], f16, tag="sm", name=f"sm{b}_{h}")
                nc.vector.tensor_tensor(
                    sm,
                    sgn[:, :, 1, :],
                    sval.unsqueeze(1).to_broadcast([P, 4, P]),
                    op=Alu.mult,
                )
                nc.vector.tensor_reduce(
                    rdiag[:, h * 4 : (h + 1) * 4], dm, axis=Ax.X, op=Alu.max
                )
                nc.vector.tensor_reduce(
                    rsel[:, h * 4 : (h + 1) * 4], sm, axis=Ax.X, op=Alu.max
                )

            cand2 = small.tile([P, NT], f16, tag="cand2", name=f"cand2_{b}")
            nc.vector.tensor_scalar(
                cand2, st[b][0], 128.0, -128.0, op0=Alu.mult, op1=Alu.add
            )
            cand2b = small.tile([P, NT], f16, tag="cand2b", name=f"cand2b{b}")
            nc.vector.tensor_tensor(cand2b, cand2, rsel, op=Alu.add)
            ans = small.tile([P, NT], f16, tag="ans", name=f"ans{b}")
            nc.vector.tensor_tensor(ans, cand2b, rdiag, op=Alu.max)
            td = small.tile([P, NT], f32, tag="td", name=f"td{b}")
            nc.vector.tensor_tensor(td, t1c, ans, op=Alu.subtract)
            tauc = small.tile([P, NT], f32, tag="tauc", name=f"tauc{b}")
            nc.vector.scalar_tensor_tensor(
                tauc, ans, 0.0, td, op0=Alu.is_gt, op1=Alu.mult
            )
            st[b] = tauc

        # tau -> transposed f16 row in xtb (SBUF->SBUF flatten DMA)
        def tau_to_xtb(b):
            psT = psD.tile([NT, P], f32, tag="psd", name=f"psTau{b}")
            nc.tensor.transpose(psT, st[b], identf)
            tauT = small.tile([NT, P], f16, tag="tauT", name=f"tauT{b}")
            nc.scalar.copy(tauT, psT)
            nc.scalar.dma_start(xtbs[b][64:65, 0:S], tauT)

        tau_to_xtb(0)

        # ---- fillers ----
        for w in range(FILL_N):
            psw = psA.tile([P, D], f32, tag="psa", name=f"fill{w}")
            nc.tensor.matmul(
                psw, lhsT=b16[:, 0:P], rhs=b16[:, 0:D], start=True, stop=True
            )

        # ---- conv per batch ----
        for b in range(BLOC):
            xta2_sb = xta2s[b]
            xtb_sb = xtbs[b]
            nc.tensor.ldweights(xtb_sb[0:65, 0:P])
            nc.tensor.ldweights(xta2_sb[:, 0:P])

            if b == BLOC - 1:
                groups = [(0, OG), (OG, 4), (OG + 4, 2), (OG + 6, 1), (OG + 7, 1)]
            else:
                groups = [(g * OG, OG) for g in range(NT // OG)]

            osbs = {}
            for i0, glen in groups:
                osb = outp.tile([P, glen, D], f16, tag=f"osb{glen}",
                                name=f"osb{b}_{i0}")
                for q in range(glen):
                    osbs[i0 + q] = (osb, q)

            for i in range(NT):
                if b + 1 < BLOC and i == OG:
                    tau_to_xtb(b + 1)
                ps = psA.tile([P, D], f32, tag="psa", name=f"ps{b}_{i}")
                nc.tensor.matmul(
                    ps,
                    lhsT=xta2_sb[:, i * P : (i + 1) * P],
                    rhs=wt12_sb,
                    start=True,
                    stop=False,
                )
                if DRAIN[i] == "A":
                    nc.tensor.matmul(
                        ps, lhsT=identh, rhs=pe_sb[:, i, :], start=False, stop=False
                    )
                nc.tensor.matmul(
                    ps,
                    lhsT=xtb_sb[0:65, i * P : (i + 1) * P],
                    rhs=wtb_sb[0:65, :],
                    start=False,
                    stop=True,
                )
                osb, q = osbs[i]
                if DRAIN[i] == "A":
                    nc.scalar.copy(osb[:, q, :], ps)
                else:
                    nc.vector.tensor_tensor(
                        osb[:, q, :], ps, pe_sb[:, i, :], op=Alu.add
                    )
                for gi, (i0, glen) in enumerate(groups):
                    if i0 + glen == i + 1:
                        dst = out[b, i0 * P : (i0 + glen) * P, :].rearrange(
                            "(q p) d -> p q d", p=P
                        )
                        nc.sync.dma_start(dst, osb)


def build_bass():
    import concourse.tile as tile
    from concourse import bacc, mybir

    f32 = mybir.dt.float32
    f16 = mybir.dt.float16

    nc = bacc.Bacc(
        "TRN2",
        target_bir_lowering=False,
        debug=False,
        enable_asserts=False,
        num_devices=NCORES,
    )
    aps = {}
    aps["xin"] = nc.dram_tensor(
        "xin", (BLOC, P, NT, C), f32, kind="ExternalInput"
    ).ap()
    aps["xta2"] = nc.dram_tensor(
        "xta2", (BLOC, P, S + 2), f16, kind="ExternalInput"
    ).ap()
    aps["xtb"] = nc.dram_tensor(
        "xtb", (BLOC, C, S + 2), f16, kind="ExternalInput"
    ).ap()
    aps["pe16"] = nc.dram_tensor("pe16", (P, NT, D), f16, kind="ExternalInput").ap()
    aps["wt12"] = nc.dram_tensor("wt12", (P, D), f16, kind="ExternalInput").ap()
    aps["wtb"] = nc.dram_tensor("wtb", (66, D), f16, kind="ExternalInput").ap()
    aps["blob16"] = nc.dram_tensor(
        "blob16", (P, B16_W), f16, kind="ExternalInput"
    ).ap()
    aps["blob32"] = nc.dram_tensor(
        "blob32", (P, B32_W), f32, kind="ExternalInput"
    ).ap()
    aps["dtmpl"] = nc.dram_tensor("dtmpl", (6, S), f16, kind="ExternalInput").ap()
    aps["t2zs"] = nc.dram_tensor(
        "t2zs", (7, NT * P), f16, kind="ExternalInput"
    ).ap()
    aps["out"] = nc.dram_tensor("out", (BLOC, S, D), f16, kind="ExternalOutput").ap()
    aps["js_dram"] = nc.dram_tensor("js_scr", (BLOC, S), f16, kind="Internal").ap()

    with tile.TileContext(nc) as tc:
        _emit(tc, aps)
    nc.compile()
    return nc


def make_consts():
    position = np.arange(S, dtype=np.float32)[:, None]
    div_term = np.exp(
        np.arange(0, D, 2, dtype=np.float32) * np.float32(-math.log(10000.0) / D)
    ).astype(np.float32)
    ang = (position * div_term).astype(np.float32)
    pe = np.zeros((S, D), dtype=np.float32)
    pe[:, 0::2] = np.sin(ang)
    pe[:, 1::2] = np.cos(ang)

    pp = np.arange(P)
    ii = np.arange(NT)
    uu = np.arange(P)
    jj = np.arange(NT)

    blob16 = np.zeros((P, B16_W), dtype=np.float16)
    blob16[:, B16_IDENT : B16_IDENT + P] = np.eye(P, dtype=np.float16)
    dval = (uu[None, None, :] < pp[:, None, None]) * (
        ii[None, :, None] * P + uu[None, None, :] + 1.0
    )
    blob16[:, B16_DVAL:B16_SVAL] = dval.reshape(P, -1).astype(np.float16)
    blob16[:, B16_SVAL:B16_MASK1] = (uu[None, :] + 1.0).astype(np.float16)
    mask1 = (jj[None, :] < ii[:, None]) * (jj[None, :] + 1.0)
    blob16[:, B16_MASK1:B16_W] = mask1.reshape(1, -1).astype(np.float16)

    blob32 = np.zeros((P, B32_W), dtype=np.float32)
    blob32[:, B32_IDENT : B32_IDENT + P] = np.eye(P, dtype=np.float32)
    blob32[:, B32_T1C : B32_T1C + NT] = (
        ii[None, :] * P + pp[:, None] + 1.0
    ).astype(np.float32)
    kc = np.array(list(range(1, 17)) + [0] + [99] * 15, dtype=np.float32)
    blob32[:, B32_KCOL] = np.concatenate([np.tile(kc, 3), [99] * 32]).astype(
        np.float32
    )[:P]
    blob32[:, B32_ONES : B32_ONES + P] = 1.0
    return pe, blob16, blob32


def make_shared_inputs(conv_w, tc_w, tc_b):
    pe, blob16, blob32 = make_consts()
    pe_b = (pe + np.asarray(tc_b, np.float32)[None, :]).astype(np.float16)
    pe_r = np.ascontiguousarray(pe_b.reshape(NT, P, D).transpose(1, 0, 2))
    wt = np.transpose(np.asarray(conv_w, np.float32), (2, 1, 0))  # (k, c, d)
    wt12 = np.concatenate([wt[1], wt[2]], axis=0).astype(np.float16)
    w = np.asarray(tc_w, np.float32)[:, 0].astype(np.float16)
    wtb = np.concatenate(
        [wt[0].astype(np.float16), w[None], np.zeros((1, D), np.float16)], axis=0
    )
    dtmpl = np.zeros((6, S), dtype=np.float16)
    dtmpl[0::2] = 1.0
    t2zs = np.ones((7, NT * P), dtype=np.float16)
    t2zs[0] = -BIG
    return {
        "pe16": pe_r,
        "wt12": np.ascontiguousarray(wt12),
        "wtb": np.ascontiguousarray(wtb),
        "blob16": blob16,
        "blob32": blob32,
        "dtmpl": dtmpl,
        "t2zs": t2zs,
    }


def make_xparts(x16):
    bl = x16.shape[0]
    xt = np.transpose(x16, (0, 2, 1))  # (bl, C, S)
    xta2 = np.zeros((bl, P, S + 2), dtype=np.float16)
    xta2[:, 0:C, 0:S] = xt                      # tap1: col k = x[k]
    xta2[:, C : 2 * C, 0 : S - 1] = xt[:, :, 1:]  # tap2: col k = x[k+1]
    xta2[:, C : 2 * C, S - 1] = xt[:, :, 0]
    xtb = np.zeros((bl, C, S + 2), dtype=np.float16)
    xtb[:, :, 1 : S + 1] = xt                   # tap0: col k = x[k-1]
    xtb[:, :, 0] = xt[:, :, S - 1]
    return xta2, xtb


_BUILD_CACHE = {}


def _install_ntff_hook():
    import sys as _sys
    import types

    if "antenv.axon_hooks" in _sys.modules:
        return
    try:
        from trn_agent_boot.trn_boot import _ntff_profile_via_ctypes

        hook = _ntff_profile_via_ctypes("/opt/axon/libaxon_pjrt.so")
        m = types.ModuleType("antenv.axon_hooks")
        m.get_axon_ntff_profile_hook = lambda: hook
        _sys.modules["antenv.axon_hooks"] = m
    except Exception as e:
        print("[kernel] ntff hook install failed:", e)


def kernel(x, conv_w, tc_w, tc_b):
    x = np.ascontiguousarray(np.asarray(x, dtype=np.float32))
    conv_w = np.asarray(conv_w, dtype=np.float32)
    tc_w = np.asarray(tc_w, dtype=np.float32)
    tc_b = np.asarray(tc_b, dtype=np.float32)
    assert x.shape == (B, S, C), x.shape

    from concourse.bass_utils import run_bass_kernel_spmd

    if "nc" not in _BUILD_CACHE:
        _BUILD_CACHE["nc"] = build_bass()
    nc = _BUILD_CACHE["nc"]

    shared = make_shared_inputs(conv_w, tc_w, tc_b)
    x16 = x.astype(np.float16)
    in_maps = []
    for c in range(NCORES):
        m = dict(shared)
        m["xin"] = np.ascontiguousarray(
            x[c * BLOC : (c + 1) * BLOC]
            .reshape(BLOC, NT, P, C)
            .transpose(0, 2, 1, 3)
        )
        xta2, xtb = make_xparts(x16[c * BLOC : (c + 1) * BLOC])
        m["xta2"] = xta2
        m["xtb"] = xtb
        in_maps.append(m)

    trace = bool(int(os.environ.get("KERNEL_TRACE", "0")))
    if trace:
        _install_ntff_hook()
    res = run_bass_kernel_spmd(
        nc, in_maps, core_ids=list(range(NCORES)), trace=trace, trace_cores=[0]
    )
    if trace and res.exec_time_ns is not None:
        print(
            f"[kernel] HW exec time: {res.exec_time_ns} ns "
            f"(mean {res.mean_exec_time_ns} ns)"
        )
        kernel.last_exec_time_ns = res.exec_time_ns
        kernel.last_trace = res.instructions_and_trace
    out = np.concatenate([r["out"] for r in res.results], axis=0).astype(np.float32)
    return out


if __name__ == "__main__":
    build_bass()
    print("build ok")


# revision 25
# speedup vs baseline: 1.0543x; 1.0543x over previous
"""Trainium2 Bass kernel for nn_DataEmbedding_v2 — v7.

Data parallel over batch (2 per core).  Output stored f16 on device (gate is
rel 2e-2 vs absmax ~1e3; f16 adds ~5e-4) and upcast on host.  The tc scan
keeps v3's exact-f32 compare algebra (f16 hi/mid/lo splits accumulated
coarse-first into f32 psum).  v7 is built around the measured ~216 ns
per-matmul floor: fewer, wider matmuls, and no DRAM roundtrip for tau.

  - select: ONE N=256 matmul per tile — diag (K=6 rows @96:102) and one-hot
    gather (K=96 rows @0:96) share lhsT; rhs is a zero-padded [102,NT,2,128]
    table so the two halves land in separate psum columns
  - conv: A = taps1+2 (one K=128 matmul vs a col-aligned double image),
    B = tap0 + tau row (K=65) — tau reaches the B lhsT via PE transpose +
    SBUF->SBUF flatten DMA (no DRAM bounce), tc_w applied as single f16 row
  - masking: ACT Sign, DVE value-mults + max-reduces (DVE/GpSimd port
    sharing avoided; GpSimd only issues DMAs/broadcasts)
  - warmup + filler matmuls keep the PE clock un-throttled
"""

import math
import os
import sys

sys.path.insert(0, "/opt/trn_rl_repo")

import numpy as np

B, S, C, D = 16, 2048, 64, 512
NCORES = 8
BLOC = B // NCORES
P = 128
NT = S // P
ETA = 0.3
EPS = 1e-08
BIG = 60000.0

WARM_N = 9           # warmup matmuls (N=512)
FILL_N = 4           # filler matmuls before conv b0
OG = 8               # output tiles per DMA group
DRAIN = ["V", "A"] * (NT // 2)   # V: DVE fused(+pe16) / A: ACT copy (pe MM)

# blob16 column layout
B16_IDENT = 0
B16_DVAL = 128
B16_SVAL = B16_DVAL + NT * P
B16_MASK1 = B16_SVAL + P
B16_W = B16_MASK1 + NT * NT
# blob32 column layout
B32_IDENT = 0
B32_T1C = 128
B32_KCOL = B32_T1C + NT
B32_ONES = B32_KCOL + 1
B32_W = B32_ONES + P


def _emit(tc, aps):
    from contextlib import ExitStack

    from concourse import mybir

    f32 = mybir.dt.float32
    f16 = mybir.dt.float16
    Alu = mybir.AluOpType
    Ax = mybir.AxisListType

    nc = tc.nc
    xin, xta2, xtb, pe16 = aps["xin"], aps["xta2"], aps["xtb"], aps["pe16"]
    wt12, wtb = aps["wt12"], aps["wtb"]
    blob16, blob32 = aps["blob16"], aps["blob32"]
    out = aps["out"]

    with ExitStack() as ctx:
        singles = ctx.enter_context(tc.tile_pool(name="singles", bufs=1))
        xpool = ctx.enter_context(tc.tile_pool(name="xpool", bufs=2))
        xtp = ctx.enter_context(tc.tile_pool(name="xtp", bufs=2))
        small = ctx.enter_context(tc.tile_pool(name="small", bufs=2))
        selp = ctx.enter_context(tc.tile_pool(name="selp", bufs=2))
        sgp = ctx.enter_context(tc.tile_pool(name="sgp", bufs=3))
        outp = ctx.enter_context(tc.tile_pool(name="outp", bufs=2))
        psA = ctx.enter_context(tc.tile_pool(name="psA", bufs=4, space="PSUM"))
        psD = ctx.enter_context(tc.tile_pool(name="psD", bufs=2, space="PSUM"))

        # ---- early DMAs (xin first for clean completion lanes) ----
        xin0 = xpool.tile([P, NT, C], f32, tag="xin", name="xin0")
        nc.scalar.dma_start(xin0, xin[0])
        xin1 = xpool.tile([P, NT, C], f32, tag="xin", name="xin1")
        nc.scalar.dma_start(xin1, xin[1])
        b16 = singles.tile([P, B16_W], f16)
        nc.scalar.dma_start(b16, blob16)
        b32 = singles.tile([P, B32_W], f32)
        nc.scalar.dma_start(b32, blob32)
        identh = b16[:, B16_IDENT : B16_IDENT + P]
        dval = b16[:, B16_DVAL:B16_SVAL].rearrange("p (i u) -> p i u", i=NT)
        sval = b16[:, B16_SVAL:B16_MASK1]
        mask1 = b16[:, B16_MASK1:B16_W].rearrange("p (i j) -> p i j", i=NT)
        identf = b32[:, B32_IDENT : B32_IDENT + P]
        t1c = b32[:, B32_T1C : B32_T1C + NT]
        kcol96 = b32[:, B32_KCOL : B32_KCOL + 1]

        xins, xta2s, xtbs = {}, {}, {}
        xins[0] = xin0
        xins[1] = xin1
        for b in range(BLOC):
            xta2_sb = xtp.tile([P, S + 2], f16, tag="xta2", name=f"xta2_{b}")
            nc.gpsimd.dma_start(xta2_sb, xta2[b])
            xta2s[b] = xta2_sb
            xtb_sb = xtp.tile([66, S + 2], f16, tag="xtb", name=f"xtb{b}")
            nc.gpsimd.dma_start(xtb_sb[0:C, :], xtb[b])
            xtbs[b] = xtb_sb

        wt12_sb = singles.tile([P, D], f16)
        nc.gpsimd.dma_start(wt12_sb, wt12)
        wtb_sb = singles.tile([66, D], f16)
        nc.gpsimd.dma_start(wtb_sb, wtb)
        pe_sb = singles.tile([P, NT, D], f16)
        for q in range(2):
            nc.gpsimd.dma_start(
                pe_sb[:, q * 8 : (q + 1) * 8, :],
                pe16[:, q * 8 : (q + 1) * 8, :],
            )

        # ---- T2z tables: one per batch, zeroed then patched ----
        t2zs = {}
        for b in range(BLOC):
            t2z = selp.tile([102, NT, 2, P], f16, tag="t2z", name=f"t2z{b}")
            nc.scalar.memzero(t2z)
            # row 16 (hi "none") = -BIG; rows 17/49/81 (q partners) = 1;
            # rows 97/99/101 (diag q partners) = 1
            nc.gpsimd.dma_start(
                t2z[16:17, :, 1, :],
                aps["t2zs"][0:1].rearrange("r (i u) -> r i u", i=NT),
            )
            nc.gpsimd.dma_start(
                t2z[17:82:32, :, 1, :],
                aps["t2zs"][1:4].rearrange("r (i u) -> r i u", i=NT),
            )
            nc.gpsimd.dma_start(
                t2z[97:102:2, :, 0, :],
                aps["t2zs"][4:7].rearrange("r (i u) -> r i u", i=NT),
            )
            t2zs[b] = t2z

        # ---- phase A, stage-interleaved across batches ----
        st = {}
        spls, negths = {}, {}
        for b in range(BLOC):
            xin_sb = xins[b]
            normc = small.tile([P, NT], f32, tag="normc", name=f"normc{b}")
            nc.vector.tensor_reduce(
                normc, xin_sb, axis=Ax.X, op=Alu.add, apply_absolute_value=True
            )
            negthc = small.tile([P, 2, NT], f32, tag="negthc", name=f"negthc{b}")
            nc.vector.tensor_scalar(negthc[:, 0, :], normc, -1.0, None, op0=Alu.mult)
            nc.vector.tensor_scalar(
                negthc[:, 1, :],
                normc,
                float(EPS),
                float(1.0 - ETA),
                op0=Alu.add,
                op1=Alu.mult,
            )
            spl = small.tile([P, 2, 3, NT], f16, tag="spl", name=f"spl{b}")
            nc.vector.tensor_copy(spl[:, :, 0, :], negthc)
            rem1 = small.tile([P, 2, NT], f32, tag="rem1", name=f"rem1_{b}")
            nc.vector.tensor_tensor(rem1, negthc, spl[:, :, 0, :], op=Alu.subtract)
            nc.vector.tensor_copy(spl[:, :, 1, :], rem1)
            rem2 = small.tile([P, 2, NT], f32, tag="rem2", name=f"rem2_{b}")
            nc.vector.tensor_tensor(rem2, rem1, spl[:, :, 1, :], op=Alu.subtract)
            nc.vector.tensor_copy(spl[:, :, 2, :], rem2)
            spls[b] = spl
            negths[b] = negthc
            ps96 = psD.tile([96, P], f16, tag="psd", name=f"ps96_{b}")
            nc.tensor.transpose(ps96, spl.rearrange("p a k j -> p (a k j)"), identh)
            spT = small.tile([96, P], f16, tag="spt", name=f"spT{b}")
            nc.scalar.copy(spT, ps96)
            psnT = psD.tile([NT, P], f32, tag="psd", name=f"psnT{b}")
            nc.tensor.transpose(psnT, negthc[:, 0, :], identf)
            bneg = small.tile([NT, 1], f32, tag="bneg", name=f"bneg{b}")
            nc.vector.tensor_reduce(bneg, psnT, axis=Ax.X, op=Alu.max)
            st[b] = [spT, bneg]
            if b == 0:
                # warmup burst rides the gap while batch 1 data lands
                for w in range(WARM_N):
                    psw = psA.tile([P, D], f32, tag="psa", name=f"warm{w}")
                    nc.tensor.matmul(
                        psw, lhsT=b16[:, 0:P], rhs=b16[:, 0:D],
                        start=True, stop=True,
                    )
        bnegbcs = {}
        for b in range(BLOC):
            spT, bneg = st[b]
            psb1 = psD.tile([1, NT], f32, tag="psd", name=f"psb1_{b}")
            nc.tensor.transpose(psb1, bneg, identf[0:NT, 0:NT])
            bnegrow = small.tile([1, NT], f32, tag="bnegrow", name=f"bnegrow{b}")
            nc.scalar.copy(bnegrow, psb1)
            psbb = psD.tile([P, NT], f32, tag="psd", name=f"psbb{b}")
            nc.tensor.matmul(
                psbb,
                lhsT=b32[0:1, B32_ONES : B32_ONES + P],
                rhs=bnegrow,
                start=True,
                stop=True,
            )
            bnegbc = small.tile([P, NT], f32, tag="bnegbc", name=f"bnegbc{b}")
            nc.scalar.copy(bnegbc, psbb)
            bnegbcs[b] = bnegbc
        for b in range(BLOC):
            spT, _ = st[b]
            negthc = negths[b]
            l1a = small.tile([P, NT, NT], f32, tag="l1a", name=f"l1a{b}")
            nc.vector.tensor_tensor(
                l1a,
                negthc[:, 1, :].unsqueeze(2).to_broadcast([P, NT, NT]),
                bnegbcs[b].unsqueeze(1).to_broadcast([P, NT, NT]),
                op=Alu.add,
            )
            l1c = small.tile([P, NT, NT], f16, tag="l1c", name=f"l1c{b}")
            nc.vector.scalar_tensor_tensor(
                l1c, l1a, 0.0, mask1, op0=Alu.is_gt, op1=Alu.mult
            )
            jp1 = small.tile([P, NT], f16, tag="jp1", name=f"jp1_{b}")
            nc.vector.tensor_reduce(jp1, l1c, axis=Ax.X, op=Alu.max)
            st[b] = (jp1, spT)
        st2 = {}
        for b in range(BLOC):
            jp1, spT = st[b]
            psj = psD.tile([NT, P], f16, tag="psd", name=f"psj{b}")
            nc.tensor.transpose(psj, jp1, identh)
            jpT = small.tile([NT, P], f16, tag="jpt", name=f"jpT{b}")
            nc.scalar.copy(jpT, psj)
            st2[b] = jpT

        # ---- select setup per batch ----
        sel = {}
        for b in range(BLOC):
            jp1, spT = st[b]
            t1 = selp.tile([102, S], f16, tag="t1", name=f"t1_{b}")
            t2z = t2zs[b]
            for g in range(3):
                nc.scalar.dma_start(
                    t2z[g * 32 : g * 32 + 16, :, 1, :],
                    spT[g * 16 : (g + 1) * 16]
                    .unsqueeze(1)
                    .to_broadcast([16, NT, P]),
                )
            for k in range(3):
                nc.scalar.dma_start(
                    t2z[96 + 2 * k : 97 + 2 * k, :, 0, :],
                    spT[16 * k : 16 * (k + 1)],
                )
            nc.scalar.dma_start(t1[96:102, :], aps["dtmpl"])
            jsrow = small.tile([1, S], f16, tag="jsrow", name=f"jsrow{b}")
            nc.sync.dma_start(jsrow, st2[b])
            ohsrc = selp.tile([96, S], f16, tag="ohsrc", name=f"ohsrc{b}")
            nc.gpsimd.partition_broadcast(ohsrc, jsrow)
            nc.vector.tensor_scalar(
                t1[0:96, :], ohsrc, kcol96[0:96], None, op0=Alu.is_equal
            )
            for k in range(3):
                nc.scalar.dma_start(
                    t1[17 + 32 * k : 18 + 32 * k, :],
                    spT[48 + 16 * k : 64 + 16 * k],
                )
            for k in range(3):
                nc.scalar.dma_start(
                    t1[97 + 2 * k : 98 + 2 * k, :],
                    spT[48 + 16 * k : 64 + 16 * k],
                )
            sel[b] = (t1, t2z)

        for b in range(BLOC):
            t1, t2z = sel[b]
            nc.tensor.ldweights(t1[:, 0:P])
            nc.tensor.ldweights(t2z[:, 0, 0, :])

        for w in range(10):
            psw = psA.tile([P, D], f32, tag="psa", name=f"fillb{w}")
            nc.tensor.matmul(
                psw, lhsT=b16[:, 0:P], rhs=b16[:, 0:D], start=True, stop=True
            )

        # ---- select matmuls + sign/value/max per batch; tau chain ----
        for b in range(BLOC):
            t1, t2z = sel[b]
            xtb_sb = xtbs[b]
            rdiag = small.tile([P, NT], f16, tag="rdiag", name=f"rdiag{b}")
            rsel = small.tile([P, NT], f16, tag="rsel", name=f"rsel{b}")
            for h in range(4):
                psd = psD.tile([P, 4, 2, P], f32, tag="psd", name=f"psd{b}_{h}")
                for ii in range(4):
                    i = h * 4 + ii
                    sl = slice(i * P, (i + 1) * P)
                    nc.tensor.matmul(
                        psd[:, ii, :, :],
                        lhsT=t1[:, sl],
                        rhs=t2z[:, i, :, :],
                        start=True,
                        stop=True,
                    )
                sgn = sgp.tile([P, 4, 2, P], f16, tag="sgn", name=f"sgn{b}_{h}")
                nc.scalar.sign(sgn, psd)
                dm = sgp.tile([P, 4, P], f16, tag="dm", name=f"dm{b}_{h}")
                nc.vector.tensor_tensor(
                    dm, sgn[:, :, 0, :], dval[:, h * 4 : (h + 1) * 4, :],
                    op=Alu.mult,
                )
                sm = sgp.tile([P, 4, P], f16, tag="sm", name=f"sm{b}_{h}")
                nc.vector.tensor_tensor(
                    sm,
                    sgn[:, :, 1, :],
                    sval.unsqueeze(1).to_broadcast([P, 4, P]),
                    op=Alu.mult,
                )
                nc.vector.tensor_reduce(
                    rdiag[:, h * 4 : (h + 1) * 4], dm, axis=Ax.X, op=Alu.max
                )
                nc.vector.tensor_reduce(
                    rsel[:, h * 4 : (h + 1) * 4], sm, axis=Ax.X, op=Alu.max
                )

            cand2 = small.tile([P, NT], f16, tag="cand2", name=f"cand2_{b}")
            nc.vector.tensor_scalar(
                cand2, st[b][0], 128.0, -128.0, op0=Alu.mult, op1=Alu.add
            )
            cand2b = small.tile([P, NT], f16, tag="cand2b", name=f"cand2b{b}")
            nc.vector.tensor_tensor(cand2b, cand2, rsel, op=Alu.add)
            ans = small.tile([P, NT], f16, tag="ans", name=f"ans{b}")
            nc.vector.tensor_tensor(ans, cand2b, rdiag, op=Alu.max)
            td = small.tile([P, NT], f32, tag="td", name=f"td{b}")
            nc.vector.tensor_tensor(td, t1c, ans, op=Alu.subtract)
            tauc = small.tile([P, NT], f32, tag="tauc", name=f"tauc{b}")
            nc.vector.scalar_tensor_tensor(
                tauc, ans, 0.0, td, op0=Alu.is_gt, op1=Alu.mult
            )
            st[b] = tauc

        # tau -> transposed f16 row in xtb (SBUF->SBUF flatten DMA)
        def tau_to_xtb(b):
            psT = psD.tile([NT, P], f32, tag="psd", name=f"psTau{b}")
            nc.tensor.transpose(psT, st[b], identf)
            tauT = small.tile([NT, P], f16, tag="tauT", name=f"tauT{b}")
            nc.scalar.copy(tauT, psT)
            nc.scalar.dma_start(xtbs[b][64:65, 0:S], tauT)

        tau_to_xtb(0)

        # ---- fillers ----
        for w in range(FILL_N):
            psw = psA.tile([P, D], f32, tag="psa", name=f"fill{w}")
            nc.tensor.matmul(
                psw, lhsT=b16[:, 0:P], rhs=b16[:, 0:D], start=True, stop=True
            )

        # ---- conv per batch ----
        for b in range(BLOC):
            xta2_sb = xta2s[b]
            xtb_sb = xtbs[b]
            nc.tensor.ldweights(xtb_sb[0:65, 0:P])
            nc.tensor.ldweights(xta2_sb[:, 0:P])

            if b == BLOC - 1:
                groups = [(0, OG), (OG, 4), (OG + 4, 2), (OG + 6, 1), (OG + 7, 1)]
            else:
                groups = [(g * OG, OG) for g in range(NT // OG)]

            osbs = {}
            for i0, glen in groups:
                osb = outp.tile([P, glen, D], f16, tag=f"osb{glen}",
                                name=f"osb{b}_{i0}")
                for q in range(glen):
                    osbs[i0 + q] = (osb, q)

            for i in range(NT):
                if b + 1 < BLOC and i == OG:
                    tau_to_xtb(b + 1)
                ps = psA.tile([P, D], f32, tag="psa", name=f"ps{b}_{i}")
                nc.tensor.matmul(
                    ps,
                    lhsT=xta2_sb[:, i * P : (i + 1) * P],
                    rhs=wt12_sb,
                    start=True,
                    stop=False,
                )
                if DRAIN[i] == "A":
                    nc.tensor.matmul(
                        ps, lhsT=identh, rhs=pe_sb[:, i, :], start=False, stop=False
                    )
                nc.tensor.matmul(
                    ps,
                    lhsT=xtb_sb[0:65, i * P : (i + 1) * P],
                    rhs=wtb_sb[0:65, :],
                    start=False,
                    stop=True,
                )
                osb, q = osbs[i]
                if DRAIN[i] == "A":
                    nc.scalar.copy(osb[:, q, :], ps)
                else:
                    nc.vector.tensor_tensor(
                        osb[:, q, :], ps, pe_sb[:, i, :], op=Alu.add
                    )
                for gi, (i0, glen) in enumerate(groups):
                    if i0 + glen == i + 1:
                        dst = out[b, i0 * P : (i0 + glen) * P, :].rearrange(
                            "(q p) d -> p q d", p=P
                        )
                        nc.sync.dma_start(dst, osb)


def build_bass():
    import concourse.tile as tile
    from concourse import bacc, mybir

    f32 = mybir.dt.float32
    f16 = mybir.dt.float16

    nc = bacc.Bacc(
        "TRN2",
        target_bir_lowering=False,
        debug=False,
        enable_asserts=False,
        num_devices=NCORES,
    )
    aps = {}
    aps["xin"] = nc.dram_tensor(
        "xin", (BLOC, P, NT, C), f32, kind="ExternalInput"
    ).ap()
    aps["xta2"] = nc.dram_tensor(
        "xta2", (BLOC, P, S + 2), f16, kind="ExternalInput"
    ).ap()
    aps["xtb"] = nc.dram_tensor(
        "xtb", (BLOC, C, S + 2), f16, kind="ExternalInput"
    ).ap()
    aps["pe16"] = nc.dram_tensor("pe16", (P, NT, D), f16, kind="ExternalInput").ap()
    aps["wt12"] = nc.dram_tensor("wt12", (P, D), f16, kind="ExternalInput").ap()
    aps["wtb"] = nc.dram_tensor("wtb", (66, D), f16, kind="ExternalInput").ap()
    aps["blob16"] = nc.dram_tensor(
        "blob16", (P, B16_W), f16, kind="ExternalInput"
    ).ap()
    aps["blob32"] = nc.dram_tensor(
        "blob32", (P, B32_W), f32, kind="ExternalInput"
    ).ap()
    aps["dtmpl"] = nc.dram_tensor("dtmpl", (6, S), f16, kind="ExternalInput").ap()
    aps["t2zs"] = nc.dram_tensor(
        "t2zs", (7, NT * P), f16, kind="ExternalInput"
    ).ap()
    aps["out"] = nc.dram_tensor("out", (BLOC, S, D), f16, kind="ExternalOutput").ap()

    with tile.TileContext(nc) as tc:
        _emit(tc, aps)
    nc.compile()
    return nc


def make_consts():
    position = np.arange(S, dtype=np.float32)[:, None]
    div_term = np.exp(
        np.arange(0, D, 2, dtype=np.float32) * np.float32(-math.log(10000.0) / D)
    ).astype(np.float32)
    ang = (position * div_term).astype(np.float32)
    pe = np.zeros((S, D), dtype=np.float32)
    pe[:, 0::2] = np.sin(ang)
    pe[:, 1::2] = np.cos(ang)

    pp = np.arange(P)
    ii = np.arange(NT)
    uu = np.arange(P)
    jj = np.arange(NT)

    blob16 = np.zeros((P, B16_W), dtype=np.float16)
    blob16[:, B16_IDENT : B16_IDENT + P] = np.eye(P, dtype=np.float16)
    dval = (uu[None, None, :] < pp[:, None, None]) * (
        ii[None, :, None] * P + uu[None, None, :] + 1.0
    )
    blob16[:, B16_DVAL:B16_SVAL] = dval.reshape(P, -1).astype(np.float16)
    blob16[:, B16_SVAL:B16_MASK1] = (uu[None, :] + 1.0).astype(np.float16)
    mask1 = (jj[None, :] < ii[:, None]) * (jj[None, :] + 1.0)
    blob16[:, B16_MASK1:B16_W] = mask1.reshape(1, -1).astype(np.float16)

    blob32 = np.zeros((P, B32_W), dtype=np.float32)
    blob32[:, B32_IDENT : B32_IDENT + P] = np.eye(P, dtype=np.float32)
    blob32[:, B32_T1C : B32_T1C + NT] = (
        ii[None, :] * P + pp[:, None] + 1.0
    ).astype(np.float32)
    kc = np.array(list(range(1, 17)) + [0] + [99] * 15, dtype=np.float32)
    blob32[:, B32_KCOL] = np.concatenate([np.tile(kc, 3), [99] * 32]).astype(
        np.float32
    )[:P]
    blob32[:, B32_ONES : B32_ONES + P] = 1.0
    return pe, blob16, blob32


def make_shared_inputs(conv_w, tc_w, tc_b):
    pe, blob16, blob32 = make_consts()
    pe_b = (pe + np.asarray(tc_b, np.float32)[None, :]).astype(np.float16)
    pe_r = np.ascontiguousarray(pe_b.reshape(NT, P, D).transpose(1, 0, 2))
    wt = np.transpose(np.asarray(conv_w, np.float32), (2, 1, 0))  # (k, c, d)
    wt12 = np.concatenate([wt[1], wt[2]], axis=0).astype(np.float16)
    w = np.asarray(tc_w, np.float32)[:, 0].astype(np.float16)
    wtb = np.concatenate(
        [wt[0].astype(np.float16), w[None], np.zeros((1, D), np.float16)], axis=0
    )
    dtmpl = np.zeros((6, S), dtype=np.float16)
    dtmpl[0::2] = 1.0
    t2zs = np.ones((7, NT * P), dtype=np.float16)
    t2zs[0] = -BIG
    return {
        "pe16": pe_r,
        "wt12": np.ascontiguousarray(wt12),
        "wtb": np.ascontiguousarray(wtb),
        "blob16": blob16,
        "blob32": blob32,
        "dtmpl": dtmpl,
        "t2zs": t2zs,
    }


def make_xparts(x16):
    bl = x16.shape[0]
    xt = np.transpose(x16, (0, 2, 1))  # (bl, C, S)
    xta2 = np.zeros((bl, P, S + 2), dtype=np.float16)
    xta2[:, 0:C, 0:S] = xt                      # tap1: col k = x[k]
    xta2[:, C : 2 * C, 0 : S - 1] = xt[:, :, 1:]  # tap2: col k = x[k+1]
    xta2[:, C : 2 * C, S - 1] = xt[:, :, 0]
    xtb = np.zeros((bl, C, S + 2), dtype=np.float16)
    xtb[:, :, 1 : S + 1] = xt                   # tap0: col k = x[k-1]
    xtb[:, :, 0] = xt[:, :, S - 1]
    return xta2, xtb


_BUILD_CACHE = {}


def _install_ntff_hook():
    import sys as _sys
    import types

    if "antenv.axon_hooks" in _sys.modules:
        return
    try:
        from trn_agent_boot.trn_boot import _ntff_profile_via_ctypes

        hook = _ntff_profile_via_ctypes("/opt/axon/libaxon_pjrt.so")
        m = types.ModuleType("antenv.axon_hooks")
        m.get_axon_ntff_profile_hook = lambda: hook
        _sys.modules["antenv.axon_hooks"] = m
    except Exception as e:
        print("[kernel] ntff hook install failed:", e)


def kernel(x, conv_w, tc_w, tc_b):
    x = np.ascontiguousarray(np.asarray(x, dtype=np.float32))
    conv_w = np.asarray(conv_w, dtype=np.float32)
    tc_w = np.asarray(tc_w, dtype=np.float32)
    tc_b = np.asarray(tc_b, dtype=np.float32)
    assert x.shape == (B, S, C), x.shape

    from concourse.bass_utils import run_bass_kernel_spmd

    if "nc" not in _BUILD_CACHE:
        _BUILD_CACHE["nc"] = build_bass()
    nc = _BUILD_CACHE["nc"]

    shared = make_shared_inputs(conv_w, tc_w, tc_b)
    x16 = x.astype(np.float16)
    in_maps = []
    for c in range(NCORES):
        m = dict(shared)
        m["xin"] = np.ascontiguousarray(
            x[c * BLOC : (c + 1) * BLOC]
            .reshape(BLOC, NT, P, C)
            .transpose(0, 2, 1, 3)
        )
        xta2, xtb = make_xparts(x16[c * BLOC : (c + 1) * BLOC])
        m["xta2"] = xta2
        m["xtb"] = xtb
        in_maps.append(m)

    trace = bool(int(os.environ.get("KERNEL_TRACE", "0")))
    if trace:
        _install_ntff_hook()
    res = run_bass_kernel_spmd(
        nc, in_maps, core_ids=list(range(NCORES)), trace=trace, trace_cores=[0]
    )
    if trace and res.exec_time_ns is not None:
        print(
            f"[kernel] HW exec time: {res.exec_time_ns} ns "
            f"(mean {res.mean_exec_time_ns} ns)"
        )
        kernel.last_exec_time_ns = res.exec_time_ns
        kernel.last_trace = res.instructions_and_trace
    out = np.concatenate([r["out"] for r in res.results], axis=0).astype(np.float32)
    return out


if __name__ == "__main__":
    build_bass()
    print("build ok")


# revision 26
# speedup vs baseline: 1.0926x; 1.0364x over previous
"""Trainium2 Bass kernel for nn_DataEmbedding_v2 — v7.

Data parallel over batch (2 per core).  Output stored f16 on device (gate is
rel 2e-2 vs absmax ~1e3; f16 adds ~5e-4) and upcast on host.  The tc scan
keeps v3's exact-f32 compare algebra (f16 hi/mid/lo splits accumulated
coarse-first into f32 psum).  v7 is built around the measured ~216 ns
per-matmul floor: fewer, wider matmuls, and no DRAM roundtrip for tau.

  - select: ONE N=256 matmul per tile — diag (K=6 rows @96:102) and one-hot
    gather (K=96 rows @0:96) share lhsT; rhs is a zero-padded [102,NT,2,128]
    table so the two halves land in separate psum columns
  - conv: A = taps1+2 (one K=128 matmul vs a col-aligned double image),
    B = tap0 + tau row (K=65) — tau reaches the B lhsT via PE transpose +
    SBUF->SBUF flatten DMA (no DRAM bounce), tc_w applied as single f16 row
  - masking: ACT Sign, DVE value-mults + max-reduces (DVE/GpSimd port
    sharing avoided; GpSimd only issues DMAs/broadcasts)
  - warmup + filler matmuls keep the PE clock un-throttled
"""

import math
import os
import sys

sys.path.insert(0, "/opt/trn_rl_repo")

import numpy as np

B, S, C, D = 16, 2048, 64, 512
NCORES = 8
BLOC = B // NCORES
P = 128
NT = S // P
ETA = 0.3
EPS = 1e-08
BIG = 60000.0

WARM_N = 9           # warmup matmuls (N=512)
FILL_N = 4           # filler matmuls before conv b0
OG = 8               # output tiles per DMA group
DRAIN = ["V", "V", "V", "A"] * (NT // 4)   # V: DVE fused(+pe16) / A: ACT copy (pe MM)

# blob16 column layout
B16_IDENT = 0
B16_DVAL = 128
B16_SVAL = B16_DVAL + NT * P
B16_MASK1 = B16_SVAL + P
B16_W = B16_MASK1 + NT * NT
# blob32 column layout
B32_IDENT = 0
B32_T1C = 128
B32_KCOL = B32_T1C + NT
B32_ONES = B32_KCOL + 1
B32_W = B32_ONES + P


def _emit(tc, aps):
    from contextlib import ExitStack

    from concourse import mybir

    f32 = mybir.dt.float32
    f16 = mybir.dt.float16
    Alu = mybir.AluOpType
    Ax = mybir.AxisListType

    nc = tc.nc
    xin, xta2, xtb, pe16 = aps["xin"], aps["xta2"], aps["xtb"], aps["pe16"]
    wt12, wtb = aps["wt12"], aps["wtb"]
    blob16, blob32 = aps["blob16"], aps["blob32"]
    out = aps["out"]

    with ExitStack() as ctx:
        singles = ctx.enter_context(tc.tile_pool(name="singles", bufs=1))
        xpool = ctx.enter_context(tc.tile_pool(name="xpool", bufs=2))
        xtp = ctx.enter_context(tc.tile_pool(name="xtp", bufs=2))
        small = ctx.enter_context(tc.tile_pool(name="small", bufs=2))
        selp = ctx.enter_context(tc.tile_pool(name="selp", bufs=2))
        sgp = ctx.enter_context(tc.tile_pool(name="sgp", bufs=3))
        outp = ctx.enter_context(tc.tile_pool(name="outp", bufs=2))
        psA = ctx.enter_context(tc.tile_pool(name="psA", bufs=4, space="PSUM"))
        psD = ctx.enter_context(tc.tile_pool(name="psD", bufs=2, space="PSUM"))

        # ---- early DMAs (xin first for clean completion lanes) ----
        xin0 = xpool.tile([P, NT, C], f32, tag="xin", name="xin0")
        nc.scalar.dma_start(xin0, xin[0])
        xin1 = xpool.tile([P, NT, C], f32, tag="xin", name="xin1")
        nc.scalar.dma_start(xin1, xin[1])
        b16 = singles.tile([P, B16_W], f16)
        nc.scalar.dma_start(b16, blob16)
        b32 = singles.tile([P, B32_W], f32)
        nc.scalar.dma_start(b32, blob32)
        identh = b16[:, B16_IDENT : B16_IDENT + P]
        dval = b16[:, B16_DVAL:B16_SVAL].rearrange("p (i u) -> p i u", i=NT)
        sval = b16[:, B16_SVAL:B16_MASK1]
        mask1 = b16[:, B16_MASK1:B16_W].rearrange("p (i j) -> p i j", i=NT)
        identf = b32[:, B32_IDENT : B32_IDENT + P]
        t1c = b32[:, B32_T1C : B32_T1C + NT]
        kcol96 = b32[:, B32_KCOL : B32_KCOL + 1]

        xins, xta2s, xtbs = {}, {}, {}
        xins[0] = xin0
        xins[1] = xin1
        for b in range(BLOC):
            xta2_sb = xtp.tile([P, S + 2], f16, tag="xta2", name=f"xta2_{b}")
            nc.gpsimd.dma_start(xta2_sb, xta2[b])
            xta2s[b] = xta2_sb
            xtb_sb = xtp.tile([66, S + 2], f16, tag="xtb", name=f"xtb{b}")
            nc.gpsimd.dma_start(xtb_sb[0:C, :], xtb[b])
            xtbs[b] = xtb_sb

        wt12_sb = singles.tile([P, D], f16)
        nc.gpsimd.dma_start(wt12_sb, wt12)
        wtb_sb = singles.tile([66, D], f16)
        nc.gpsimd.dma_start(wtb_sb, wtb)
        pe_sb = singles.tile([P, NT, D], f16)
        for q in range(2):
            nc.gpsimd.dma_start(
                pe_sb[:, q * 8 : (q + 1) * 8, :],
                pe16[:, q * 8 : (q + 1) * 8, :],
            )

        # ---- T2z tables: one per batch, zeroed then patched ----
        t2zs = {}
        for b in range(BLOC):
            t2z = selp.tile([102, NT, 2, P], f16, tag="t2z", name=f"t2z{b}")
            nc.scalar.memzero(t2z)
            # row 16 (hi "none") = -BIG; rows 17/49/81 (q partners) = 1;
            # rows 97/99/101 (diag q partners) = 1
            nc.gpsimd.dma_start(
                t2z[16:17, :, 1, :],
                aps["t2zs"][0:1].rearrange("r (i u) -> r i u", i=NT),
            )
            nc.gpsimd.dma_start(
                t2z[17:82:32, :, 1, :],
                aps["t2zs"][1:4].rearrange("r (i u) -> r i u", i=NT),
            )
            nc.gpsimd.dma_start(
                t2z[97:102:2, :, 0, :],
                aps["t2zs"][4:7].rearrange("r (i u) -> r i u", i=NT),
            )
            t2zs[b] = t2z

        # ---- phase A, stage-interleaved across batches ----
        st = {}
        spls, negths = {}, {}
        for b in range(BLOC):
            xin_sb = xins[b]
            normc = small.tile([P, NT], f32, tag="normc", name=f"normc{b}")
            nc.vector.tensor_reduce(
                normc, xin_sb, axis=Ax.X, op=Alu.add, apply_absolute_value=True
            )
            negthc = small.tile([P, 2, NT], f32, tag="negthc", name=f"negthc{b}")
            nc.vector.tensor_scalar(negthc[:, 0, :], normc, -1.0, None, op0=Alu.mult)
            nc.vector.tensor_scalar(
                negthc[:, 1, :],
                normc,
                float(EPS),
                float(1.0 - ETA),
                op0=Alu.add,
                op1=Alu.mult,
            )
            spl = small.tile([P, 2, 3, NT], f16, tag="spl", name=f"spl{b}")
            nc.vector.tensor_copy(spl[:, :, 0, :], negthc)
            rem1 = small.tile([P, 2, NT], f32, tag="rem1", name=f"rem1_{b}")
            nc.vector.tensor_tensor(rem1, negthc, spl[:, :, 0, :], op=Alu.subtract)
            nc.vector.tensor_copy(spl[:, :, 1, :], rem1)
            rem2 = small.tile([P, 2, NT], f32, tag="rem2", name=f"rem2_{b}")
            nc.vector.tensor_tensor(rem2, rem1, spl[:, :, 1, :], op=Alu.subtract)
            nc.vector.tensor_copy(spl[:, :, 2, :], rem2)
            spls[b] = spl
            negths[b] = negthc
            ps96 = psD.tile([96, P], f16, tag="psd", name=f"ps96_{b}")
            nc.tensor.transpose(ps96, spl.rearrange("p a k j -> p (a k j)"), identh)
            spT = small.tile([96, P], f16, tag="spt", name=f"spT{b}")
            nc.scalar.copy(spT, ps96)
            psnT = psD.tile([NT, P], f32, tag="psd", name=f"psnT{b}")
            nc.tensor.transpose(psnT, negthc[:, 0, :], identf)
            bneg = small.tile([NT, 1], f32, tag="bneg", name=f"bneg{b}")
            nc.vector.tensor_reduce(bneg, psnT, axis=Ax.X, op=Alu.max)
            st[b] = [spT, bneg]
            if b == 0:
                # warmup burst rides the gap while batch 1 data lands
                for w in range(WARM_N):
                    psw = psA.tile([P, D], f32, tag="psa", name=f"warm{w}")
                    nc.tensor.matmul(
                        psw, lhsT=b16[:, 0:P], rhs=b16[:, 0:D],
                        start=True, stop=True,
                    )
        bnegbcs = {}
        for b in range(BLOC):
            spT, bneg = st[b]
            psb1 = psD.tile([1, NT], f32, tag="psd", name=f"psb1_{b}")
            nc.tensor.transpose(psb1, bneg, identf[0:NT, 0:NT])
            bnegrow = small.tile([1, NT], f32, tag="bnegrow", name=f"bnegrow{b}")
            nc.scalar.copy(bnegrow, psb1)
            psbb = psD.tile([P, NT], f32, tag="psd", name=f"psbb{b}")
            nc.tensor.matmul(
                psbb,
                lhsT=b32[0:1, B32_ONES : B32_ONES + P],
                rhs=bnegrow,
                start=True,
                stop=True,
            )
            bnegbc = small.tile([P, NT], f32, tag="bnegbc", name=f"bnegbc{b}")
            nc.scalar.copy(bnegbc, psbb)
            bnegbcs[b] = bnegbc
        for b in range(BLOC):
            spT, _ = st[b]
            negthc = negths[b]
            l1a = small.tile([P, NT, NT], f32, tag="l1a", name=f"l1a{b}")
            nc.vector.tensor_tensor(
                l1a,
                negthc[:, 1, :].unsqueeze(2).to_broadcast([P, NT, NT]),
                bnegbcs[b].unsqueeze(1).to_broadcast([P, NT, NT]),
                op=Alu.add,
            )
            l1c = small.tile([P, NT, NT], f16, tag="l1c", name=f"l1c{b}")
            nc.vector.scalar_tensor_tensor(
                l1c, l1a, 0.0, mask1, op0=Alu.is_gt, op1=Alu.mult
            )
            jp1 = small.tile([P, NT], f16, tag="jp1", name=f"jp1_{b}")
            nc.vector.tensor_reduce(jp1, l1c, axis=Ax.X, op=Alu.max)
            st[b] = (jp1, spT)
        st2 = {}
        for b in range(BLOC):
            jp1, spT = st[b]
            psj = psD.tile([NT, P], f16, tag="psd", name=f"psj{b}")
            nc.tensor.transpose(psj, jp1, identh)
            jpT = small.tile([NT, P], f16, tag="jpt", name=f"jpT{b}")
            nc.scalar.copy(jpT, psj)
            st2[b] = jpT

        # ---- select setup per batch ----
        sel = {}
        for b in range(BLOC):
            jp1, spT = st[b]
            t1 = selp.tile([102, S], f16, tag="t1", name=f"t1_{b}")
            t2z = t2zs[b]
            for g in range(3):
                nc.scalar.dma_start(
                    t2z[g * 32 : g * 32 + 16, :, 1, :],
                    spT[g * 16 : (g + 1) * 16]
                    .unsqueeze(1)
                    .to_broadcast([16, NT, P]),
                )
            for k in range(3):
                nc.scalar.dma_start(
                    t2z[96 + 2 * k : 97 + 2 * k, :, 0, :],
                    spT[16 * k : 16 * (k + 1)],
                )
            nc.scalar.dma_start(t1[96:102, :], aps["dtmpl"])
            jsrow = small.tile([1, S], f16, tag="jsrow", name=f"jsrow{b}")
            nc.sync.dma_start(jsrow, st2[b])
            ohsrc = selp.tile([96, S], f16, tag="ohsrc", name=f"ohsrc{b}")
            nc.gpsimd.partition_broadcast(ohsrc, jsrow)
            nc.vector.tensor_scalar(
                t1[0:96, :], ohsrc, kcol96[0:96], None, op0=Alu.is_equal
            )
            for k in range(3):
                nc.scalar.dma_start(
                    t1[17 + 32 * k : 18 + 32 * k, :],
                    spT[48 + 16 * k : 64 + 16 * k],
                )
            for k in range(3):
                nc.scalar.dma_start(
                    t1[97 + 2 * k : 98 + 2 * k, :],
                    spT[48 + 16 * k : 64 + 16 * k],
                )
            sel[b] = (t1, t2z)

        for b in range(BLOC):
            t1, t2z = sel[b]
            nc.tensor.ldweights(t1[:, 0:P])
            nc.tensor.ldweights(t2z[:, 0, 0, :])

        for w in range(10):
            psw = psA.tile([P, D], f32, tag="psa", name=f"fillb{w}")
            nc.tensor.matmul(
                psw, lhsT=b16[:, 0:P], rhs=b16[:, 0:D], start=True, stop=True
            )

        # ---- select matmuls + sign/value/max per batch; tau chain ----
        for b in range(BLOC):
            t1, t2z = sel[b]
            xtb_sb = xtbs[b]
            rdiag = small.tile([P, NT], f16, tag="rdiag", name=f"rdiag{b}")
            rsel = small.tile([P, NT], f16, tag="rsel", name=f"rsel{b}")
            for h in range(4):
                psd = psD.tile([P, 4, 2, P], f32, tag="psd", name=f"psd{b}_{h}")
                for ii in range(4):
                    i = h * 4 + ii
                    sl = slice(i * P, (i + 1) * P)
                    nc.tensor.matmul(
                        psd[:, ii, :, :],
                        lhsT=t1[:, sl],
                        rhs=t2z[:, i, :, :],
                        start=True,
                        stop=True,
                    )
                sgn = sgp.tile([P, 4, 2, P], f16, tag="sgn", name=f"sgn{b}_{h}")
                nc.scalar.sign(sgn, psd)
                dm = sgp.tile([P, 4, P], f16, tag="dm", name=f"dm{b}_{h}")
                nc.vector.tensor_tensor(
                    dm, sgn[:, :, 0, :], dval[:, h * 4 : (h + 1) * 4, :],
                    op=Alu.mult,
                )
                sm = sgp.tile([P, 4, P], f16, tag="sm", name=f"sm{b}_{h}")
                nc.vector.tensor_tensor(
                    sm,
                    sgn[:, :, 1, :],
                    sval.unsqueeze(1).to_broadcast([P, 4, P]),
                    op=Alu.mult,
                )
                nc.vector.tensor_reduce(
                    rdiag[:, h * 4 : (h + 1) * 4], dm, axis=Ax.X, op=Alu.max
                )
                nc.vector.tensor_reduce(
                    rsel[:, h * 4 : (h + 1) * 4], sm, axis=Ax.X, op=Alu.max
                )

            cand2 = small.tile([P, NT], f16, tag="cand2", name=f"cand2_{b}")
            nc.vector.tensor_scalar(
                cand2, st[b][0], 128.0, -128.0, op0=Alu.mult, op1=Alu.add
            )
            cand2b = small.tile([P, NT], f16, tag="cand2b", name=f"cand2b{b}")
            nc.vector.tensor_tensor(cand2b, cand2, rsel, op=Alu.add)
            ans = small.tile([P, NT], f16, tag="ans", name=f"ans{b}")
            nc.vector.tensor_tensor(ans, cand2b, rdiag, op=Alu.max)
            td = small.tile([P, NT], f32, tag="td", name=f"td{b}")
            nc.vector.tensor_tensor(td, t1c, ans, op=Alu.subtract)
            tauc = small.tile([P, NT], f32, tag="tauc", name=f"tauc{b}")
            nc.vector.scalar_tensor_tensor(
                tauc, ans, 0.0, td, op0=Alu.is_gt, op1=Alu.mult
            )
            st[b] = tauc

        # tau -> transposed f16 row in xtb (SBUF->SBUF flatten DMA)
        def tau_to_xtb(b):
            psT = psD.tile([NT, P], f32, tag="psd", name=f"psTau{b}")
            nc.tensor.transpose(psT, st[b], identf)
            tauT = small.tile([NT, P], f16, tag="tauT", name=f"tauT{b}")
            nc.scalar.copy(tauT, psT)
            nc.scalar.dma_start(xtbs[b][64:65, 0:S], tauT)

        tau_to_xtb(0)

        # ---- fillers ----
        for w in range(FILL_N):
            psw = psA.tile([P, D], f32, tag="psa", name=f"fill{w}")
            nc.tensor.matmul(
                psw, lhsT=b16[:, 0:P], rhs=b16[:, 0:D], start=True, stop=True
            )

        # ---- conv per batch ----
        for b in range(BLOC):
            xta2_sb = xta2s[b]
            xtb_sb = xtbs[b]
            nc.tensor.ldweights(xtb_sb[0:65, 0:P])
            nc.tensor.ldweights(xta2_sb[:, 0:P])

            if b == BLOC - 1:
                groups = [(0, OG), (OG, 4), (OG + 4, 2), (OG + 6, 1), (OG + 7, 1)]
            else:
                groups = [(g * OG, OG) for g in range(NT // OG)]

            osbs = {}
            for i0, glen in groups:
                osb = outp.tile([P, glen, D], f16, tag=f"osb{glen}",
                                name=f"osb{b}_{i0}")
                for q in range(glen):
                    osbs[i0 + q] = (osb, q)

            for i in range(NT):
                if b + 1 < BLOC and i == OG:
                    tau_to_xtb(b + 1)
                ps = psA.tile([P, D], f32, tag="psa", name=f"ps{b}_{i}")
                nc.tensor.matmul(
                    ps,
                    lhsT=xta2_sb[:, i * P : (i + 1) * P],
                    rhs=wt12_sb,
                    start=True,
                    stop=False,
                )
                if DRAIN[i] == "A":
                    nc.tensor.matmul(
                        ps, lhsT=identh, rhs=pe_sb[:, i, :], start=False, stop=False
                    )
                nc.tensor.matmul(
                    ps,
                    lhsT=xtb_sb[0:65, i * P : (i + 1) * P],
                    rhs=wtb_sb[0:65, :],
                    start=False,
                    stop=True,
                )
                osb, q = osbs[i]
                if DRAIN[i] == "A":
                    nc.scalar.copy(osb[:, q, :], ps)
                else:
                    nc.vector.tensor_tensor(
                        osb[:, q, :], ps, pe_sb[:, i, :], op=Alu.add
                    )
                for gi, (i0, glen) in enumerate(groups):
                    if i0 + glen == i + 1:
                        dst = out[b, i0 * P : (i0 + glen) * P, :].rearrange(
                            "(q p) d -> p q d", p=P
                        )
                        nc.sync.dma_start(dst, osb)


def build_bass():
    import concourse.tile as tile
    from concourse import bacc, mybir

    f32 = mybir.dt.float32
    f16 = mybir.dt.float16

    nc = bacc.Bacc(
        "TRN2",
        target_bir_lowering=False,
        debug=False,
        enable_asserts=False,
        num_devices=NCORES,
    )
    aps = {}
    aps["xin"] = nc.dram_tensor(
        "xin", (BLOC, P, NT, C), f32, kind="ExternalInput"
    ).ap()
    aps["xta2"] = nc.dram_tensor(
        "xta2", (BLOC, P, S + 2), f16, kind="ExternalInput"
    ).ap()
    aps["xtb"] = nc.dram_tensor(
        "xtb", (BLOC, C, S + 2), f16, kind="ExternalInput"
    ).ap()
    aps["pe16"] = nc.dram_tensor("pe16", (P, NT, D), f16, kind="ExternalInput").ap()
    aps["wt12"] = nc.dram_tensor("wt12", (P, D), f16, kind="ExternalInput").ap()
    aps["wtb"] = nc.dram_tensor("wtb", (66, D), f16, kind="ExternalInput").ap()
    aps["blob16"] = nc.dram_tensor(
        "blob16", (P, B16_W), f16, kind="ExternalInput"
    ).ap()
    aps["blob32"] = nc.dram_tensor(
        "blob32", (P, B32_W), f32, kind="ExternalInput"
    ).ap()
    aps["dtmpl"] = nc.dram_tensor("dtmpl", (6, S), f16, kind="ExternalInput").ap()
    aps["t2zs"] = nc.dram_tensor(
        "t2zs", (7, NT * P), f16, kind="ExternalInput"
    ).ap()
    aps["out"] = nc.dram_tensor("out", (BLOC, S, D), f16, kind="ExternalOutput").ap()

    with tile.TileContext(nc) as tc:
        _emit(tc, aps)
    nc.compile()
    return nc


def make_consts():
    position = np.arange(S, dtype=np.float32)[:, None]
    div_term = np.exp(
        np.arange(0, D, 2, dtype=np.float32) * np.float32(-math.log(10000.0) / D)
    ).astype(np.float32)
    ang = (position * div_term).astype(np.float32)
    pe = np.zeros((S, D), dtype=np.float32)
    pe[:, 0::2] = np.sin(ang)
    pe[:, 1::2] = np.cos(ang)

    pp = np.arange(P)
    ii = np.arange(NT)
    uu = np.arange(P)
    jj = np.arange(NT)

    blob16 = np.zeros((P, B16_W), dtype=np.float16)
    blob16[:, B16_IDENT : B16_IDENT + P] = np.eye(P, dtype=np.float16)
    dval = (uu[None, None, :] < pp[:, None, None]) * (
        ii[None, :, None] * P + uu[None, None, :] + 1.0
    )
    blob16[:, B16_DVAL:B16_SVAL] = dval.reshape(P, -1).astype(np.float16)
    blob16[:, B16_SVAL:B16_MASK1] = (uu[None, :] + 1.0).astype(np.float16)
    mask1 = (jj[None, :] < ii[:, None]) * (jj[None, :] + 1.0)
    blob16[:, B16_MASK1:B16_W] = mask1.reshape(1, -1).astype(np.float16)

    blob32 = np.zeros((P, B32_W), dtype=np.float32)
    blob32[:, B32_IDENT : B32_IDENT + P] = np.eye(P, dtype=np.float32)
    blob32[:, B32_T1C : B32_T1C + NT] = (
        ii[None, :] * P + pp[:, None] + 1.0
    ).astype(np.float32)
    kc = np.array(list(range(1, 17)) + [0] + [99] * 15, dtype=np.float32)
    blob32[:, B32_KCOL] = np.concatenate([np.tile(kc, 3), [99] * 32]).astype(
        np.float32
    )[:P]
    blob32[:, B32_ONES : B32_ONES + P] = 1.0
    return pe, blob16, blob32


def make_shared_inputs(conv_w, tc_w, tc_b):
    pe, blob16, blob32 = make_consts()
    pe_b = (pe + np.asarray(tc_b, np.float32)[None, :]).astype(np.float16)
    pe_r = np.ascontiguousarray(pe_b.reshape(NT, P, D).transpose(1, 0, 2))
    wt = np.transpose(np.asarray(conv_w, np.float32), (2, 1, 0))  # (k, c, d)
    wt12 = np.concatenate([wt[1], wt[2]], axis=0).astype(np.float16)
    w = np.asarray(tc_w, np.float32)[:, 0].astype(np.float16)
    wtb = np.concatenate(
        [wt[0].astype(np.float16), w[None], np.zeros((1, D), np.float16)], axis=0
    )
    dtmpl = np.zeros((6, S), dtype=np.float16)
    dtmpl[0::2] = 1.0
    t2zs = np.ones((7, NT * P), dtype=np.float16)
    t2zs[0] = -BIG
    return {
        "pe16": pe_r,
        "wt12": np.ascontiguousarray(wt12),
        "wtb": np.ascontiguousarray(wtb),
        "blob16": blob16,
        "blob32": blob32,
        "dtmpl": dtmpl,
        "t2zs": t2zs,
    }


def make_xparts(x16):
    bl = x16.shape[0]
    xt = np.transpose(x16, (0, 2, 1))  # (bl, C, S)
    xta2 = np.zeros((bl, P, S + 2), dtype=np.float16)
    xta2[:, 0:C, 0:S] = xt                      # tap1: col k = x[k]
    xta2[:, C : 2 * C, 0 : S - 1] = xt[:, :, 1:]  # tap2: col k = x[k+1]
    xta2[:, C : 2 * C, S - 1] = xt[:, :, 0]
    xtb = np.zeros((bl, C, S + 2), dtype=np.float16)
    xtb[:, :, 1 : S + 1] = xt                   # tap0: col k = x[k-1]
    xtb[:, :, 0] = xt[:, :, S - 1]
    return xta2, xtb


_BUILD_CACHE = {}


def _install_ntff_hook():
    import sys as _sys
    import types

    if "antenv.axon_hooks" in _sys.modules:
        return
    try:
        from trn_agent_boot.trn_boot import _ntff_profile_via_ctypes

        hook = _ntff_profile_via_ctypes("/opt/axon/libaxon_pjrt.so")
        m = types.ModuleType("antenv.axon_hooks")
        m.get_axon_ntff_profile_hook = lambda: hook
        _sys.modules["antenv.axon_hooks"] = m
    except Exception as e:
        print("[kernel] ntff hook install failed:", e)


def kernel(x, conv_w, tc_w, tc_b):
    x = np.ascontiguousarray(np.asarray(x, dtype=np.float32))
    conv_w = np.asarray(conv_w, dtype=np.float32)
    tc_w = np.asarray(tc_w, dtype=np.float32)
    tc_b = np.asarray(tc_b, dtype=np.float32)
    assert x.shape == (B, S, C), x.shape

    from concourse.bass_utils import run_bass_kernel_spmd

    if "nc" not in _BUILD_CACHE:
        _BUILD_CACHE["nc"] = build_bass()
    nc = _BUILD_CACHE["nc"]

    shared = make_shared_inputs(conv_w, tc_w, tc_b)
    x16 = x.astype(np.float16)
    in_maps = []
    for c in range(NCORES):
        m = dict(shared)
        m["xin"] = np.ascontiguousarray(
            x[c * BLOC : (c + 1) * BLOC]
            .reshape(BLOC, NT, P, C)
            .transpose(0, 2, 1, 3)
        )
        xta2, xtb = make_xparts(x16[c * BLOC : (c + 1) * BLOC])
        m["xta2"] = xta2
        m["xtb"] = xtb
        in_maps.append(m)

    trace = bool(int(os.environ.get("KERNEL_TRACE", "0")))
    if trace:
        _install_ntff_hook()
    res = run_bass_kernel_spmd(
        nc, in_maps, core_ids=list(range(NCORES)), trace=trace, trace_cores=[0]
    )
    if trace and res.exec_time_ns is not None:
        print(
            f"[kernel] HW exec time: {res.exec_time_ns} ns "
            f"(mean {res.mean_exec_time_ns} ns)"
        )
        kernel.last_exec_time_ns = res.exec_time_ns
        kernel.last_trace = res.instructions_and_trace
    out = np.concatenate([r["out"] for r in res.results], axis=0).astype(np.float32)
    return out


if __name__ == "__main__":
    build_bass()
    print("build ok")
